# revision 1
# baseline (speedup 1.0000x reference)
"""CrossAttentionFusion kernel for Trainium2 (8 NeuronCores, data-parallel over batch).

Reference computation (per batch element, S=2048, D=512, HID=256):
  Q = l @ Wq + bq ; K = a @ Wk + bk ; V = a @ Wv + bv
  P = softmax(Q K^T / sqrt(D)) ; O = P @ V
  fused_l = gl*O + (2-gl)*l          (gl = sigmoid(alpha_l))
  fused_a = (1+ga)*a                 (ga = sigmoid(alpha_a))
  w = sigmoid(relu(v @ W1 + b1) @ W2 + b2) ; fused_v = w*v
  out = concat([fused_l, fused_a, fused_v], -1)     # [S, 3D]

Kernel strategy (per core, one batch element):
  - scores are bounded (|s| <~ 10 after the 1/sqrt(D) scaling), so softmax skips
    the max-subtraction pass: P = exp(s), then O = (P@V) / rowsum. The rowsum is
    produced by the same PV matmuls via a ones-column appended to V (split into
    N=256 and N=257 matmuls to stay within one PSUM bank each).
  - all matmuls run in bf16 with fp32 PSUM accumulation; elementwise epilogues
    (residuals, gates) read the original fp32 activations.
  - x^T layouts ([d, s]) are produced on TensorE by matmuls against a bf16
    identity (normal matmul mode, which keeps the HAM clock warm).
  - the pre-attention work is streamed per 512-row s-chunk (loads, casts,
    transposes, projections interleaved) so TensorE stays dense and warm.
"""

import math
from contextlib import ExitStack

import ml_dtypes
import numpy as np

import concourse.bass as bass
import concourse.tile as tile
from concourse import bacc, mybir
from concourse.bass_utils import run_bass_kernel_spmd

B, S, D = 8, 2048, 512
HID = D // 2
P = 128  # partitions
NS = S // P          # 16 s-tiles
NC = D // P          # 4 d-chunks
NH = HID // P        # 2 hid-chunks
QB = 512             # q-block / s-chunk size
NQB = S // QB        # 4 chunks
TPC = QB // P        # 4 s-tiles per chunk
SCALE = 1.0 / math.sqrt(D)
DV = D + 1           # V width incl. ones column
N1 = 256             # PV split sizes
N2 = DV - N1         # 257

F32 = mybir.dt.float32
BF16 = mybir.dt.bfloat16


def build_kernel(gl: float, ga: float, b2val: float):
    nc = bacc.Bacc("TRN2", target_bir_lowering=False, debug=False, num_devices=8)

    a_t = nc.dram_tensor("a_t", [NS, P, D], F32, kind="ExternalInput").ap()
    l_t = nc.dram_tensor("l_t", [NS, P, D], F32, kind="ExternalInput").ap()
    v_t = nc.dram_tensor("v_t", [NS, P, D], F32, kind="ExternalInput").ap()
    wq = nc.dram_tensor("wq", [NC, P, D], F32, kind="ExternalInput").ap()
    wk = nc.dram_tensor("wk", [NC, P, D], F32, kind="ExternalInput").ap()
    wv = nc.dram_tensor("wv", [NC, P, D], F32, kind="ExternalInput").ap()
    w1 = nc.dram_tensor("w1", [NC, P, HID], F32, kind="ExternalInput").ap()
    w2 = nc.dram_tensor("w2", [P, NH], F32, kind="ExternalInput").ap()
    bq = nc.dram_tensor("bq", [P, NC], F32, kind="ExternalInput").ap()
    bk = nc.dram_tensor("bk", [P, NC], F32, kind="ExternalInput").ap()
    bv = nc.dram_tensor("bv", [1, D], F32, kind="ExternalInput").ap()
    b1 = nc.dram_tensor("b1", [P, NH], F32, kind="ExternalInput").ap()
    ident_in = nc.dram_tensor("ident_in", [P, P], BF16, kind="ExternalInput").ap()
    out = nc.dram_tensor("out", [NS, P, 3 * D], F32, kind="ExternalOutput").ap()

    with tile.TileContext(nc) as tc:
        _emit(tc, a_t, l_t, v_t, wq, wk, wv, w1, w2, bq, bk, bv, b1, ident_in,
              out, gl, ga, b2val)

    nc.compile()
    return nc


def _emit(tc, a_t, l_t, v_t, wq, wk, wv, w1, w2, bq, bk, bv, b1, ident_in, out, gl, ga, b2val):
    nc = tc.nc
    AF = mybir.ActivationFunctionType
    OP = mybir.AluOpType

    ctx = ExitStack()
    consts = ctx.enter_context(tc.tile_pool(name="consts", bufs=1))
    persist = ctx.enter_context(tc.tile_pool(name="persist", bufs=1))
    stage = ctx.enter_context(tc.tile_pool(name="stage", bufs=2))
    psum_mm = ctx.enter_context(tc.tile_pool(name="psum_mm", bufs=4, space="PSUM"))

    # ---- constants ----
    ident = consts.tile([P, P], BF16, tag="ident")
    nc.sync.dma_start(out=ident[:], in_=ident_in)

    # HAM warm-up: dependency-free matmuls so the PE clock gate opens (4/8 ->
    # 8/8) while the first DMAs are still streaming in.
    warm_in = consts.tile([P, P], BF16, tag="warm_in")
    nc.vector.memset(warm_in[:], 0.5)
    with tc.tile_pool(name="psum_warm", bufs=1, space="PSUM") as psum_warm:
        wps = psum_warm.tile([P, P], F32, tag="warm")
        for _ in range(176):
            nc.tensor.matmul(
                wps[:], lhsT=warm_in[:], rhs=warm_in[:], start=True, stop=True
            )

    # chunk-0 v/l bf16 cast-loads go FIRST on the SWDGE queue so the first
    # transposes aren't stuck behind bias/weight DMAs.
    def load_chunk_bf(dram, sc, tag):
        """One SWDGE cast-DMA: fp32 DRAM chunk -> bf16 [P, TPC, D] tile."""
        t = stage.tile([P, TPC, D], BF16, tag=tag, bufs=2, name=f"{tag}{sc}")
        src_ap = dram[sc * TPC : (sc + 1) * TPC].rearrange("t p d -> p t d")
        nc.gpsimd.dma_start(out=t[:], in_=src_ap)
        return t

    # chunk 0 takes the HWDGE-f32 + DVE-cast path instead: the sync ring and
    # DVE are live several us before the Q7 SWDGE queue.
    pre = {}
    for st4 in range(TPC):
        for nm, dram in (("v", v_t), ("l", l_t)):
            f = stage.tile([P, D], F32, tag="pre_f32", bufs=8, name=f"pre{nm}{st4}")
            nc.sync.dma_start(out=f[:], in_=dram[st4])
            pre[(nm, st4)] = f

    # biases: one DMA each ([P, n] layouts prepared host-side)
    bq_sb = consts.tile([P, NC], F32, tag="bq_sb")
    bk_sb = consts.tile([P, NC], F32, tag="bk_sb")
    b1_sb = consts.tile([P, NH], F32, tag="b1_sb")
    nc.gpsimd.dma_start(out=bq_sb[:], in_=bq)
    nc.gpsimd.dma_start(out=bk_sb[:], in_=bk)
    nc.gpsimd.dma_start(out=b1_sb[:], in_=b1)
    bv_bc = consts.tile([P, D], F32, tag="bv_bc")
    bv_bcast_ap = bass.AP(tensor=bv.tensor, offset=bv.offset, ap=[[0, P], bv.ap[1]])
    nc.gpsimd.dma_start(out=bv_bc[:], in_=bv_bcast_ap)

    # weights: fp32->bf16 cast happens inside the SWDGE DMA (no DVE pass)
    wq_bf = consts.tile([P, NC, D], BF16, tag="wq_bf")
    wk_bf = consts.tile([P, NC, D], BF16, tag="wk_bf")
    wv_bf = consts.tile([P, NC, D], BF16, tag="wv_bf")
    w1_bf = consts.tile([P, NC, HID], BF16, tag="w1_bf")
    w2_bf = consts.tile([P, NH], BF16, tag="w2_bf")
    for dram, sb, nchunks in ((wk, wk_bf, NC), (wv, wv_bf, NC), (w1, w1_bf, NC)):
        for c in range(nchunks):
            nc.gpsimd.dma_start(out=sb[:, c, :], in_=dram[c])
    nc.gpsimd.dma_start(out=w2_bf[:], in_=w2)
    for c in range(NC):
        nc.gpsimd.dma_start(out=wq_bf[:, c, :], in_=wq[c])

    # ---- persistent activations ----
    kT = persist.tile([P, NC, S], BF16, tag="kT")        # K^T [d, s]
    qT = persist.tile([P, NC, S], BF16, tag="qT")        # Q^T [d, s]
    v_sb = persist.tile([P, NS, DV], BF16, tag="v_sb")   # [V | 1] natural bf16
    w_sb = persist.tile([P, NS], F32, tag="w_sb")        # visual weight per s-tile
    nc.vector.memset(v_sb[:, :, D:DV], 1.0)              # ones column

    def transpose_tile(bf, xT_c, st4, on_act):
        """Write transpose of bf16 [P, D] tile into xT_c[:, :, st4*P:(st4+1)*P]
        via PE identity matmuls."""
        ps = psum_mm.tile([P, NC * P], F32, tag="mm")
        for c in range(NC):
            nc.tensor.matmul(
                ps[:, c * P : (c + 1) * P],
                lhsT=bf[:, c * P : (c + 1) * P],
                rhs=ident[:],
                start=True,
                stop=True,
            )
        dst = xT_c[:, :, st4 * P : (st4 + 1) * P]
        if on_act:
            nc.scalar.copy(dst, ps[:])
        else:
            nc.vector.tensor_copy(dst, ps[:])

    # ---- streaming phase: per s-chunk loads + transposes + projections ----
    with (
        tc.tile_pool(name="chunkT", bufs=2) as cpool,
        tc.tile_pool(name="psum_w", bufs=2, space="PSUM") as psum_w,
    ):
        for sc in range(NQB):
            aT = cpool.tile([P, NC, QB], BF16, tag="aT")
            lT = cpool.tile([P, NC, QB], BF16, tag="lT")
            vT = cpool.tile([P, NC, QB], BF16, tag="vT")
            hT = cpool.tile([P, NH, QB], BF16, tag="hT")
            if sc > 0:
                v_bf = load_chunk_bf(v_t, sc, "v_bf")
                l_bf = load_chunk_bf(l_t, sc, "l_bf")
            for st4 in range(TPC):
                st = sc * TPC + st4
                if sc == 0:
                    vbf = stage.tile([P, D], BF16, tag="in_bf", bufs=4)
                    nc.vector.tensor_copy(vbf[:], pre[("v", st4)][:])
                else:
                    vbf = v_bf[:, st4, :]
                transpose_tile(vbf, vT, st4, on_act=True)
                af = stage.tile([P, D], F32, tag="a_f32", bufs=4, name=f"af{st}")
                nc.sync.dma_start(out=af[:], in_=a_t[st])
                abf = stage.tile([P, D], BF16, tag="in_bf", bufs=4)
                nc.vector.tensor_copy(abf[:], af[:])
                transpose_tile(abf, aT, st4, on_act=False)
                oa = stage.tile([P, D], F32, tag="out_a", bufs=3)
                nc.vector.tensor_scalar_mul(out=oa[:], in0=af[:], scalar1=1.0 + ga)
                nc.scalar.dma_start(out=out[st, :, D : 2 * D], in_=oa[:])
                if sc == 0:
                    lbf = stage.tile([P, D], BF16, tag="in_bf", bufs=4)
                    nc.vector.tensor_copy(lbf[:], pre[("l", st4)][:])
                else:
                    lbf = l_bf[:, st4, :]
                transpose_tile(lbf, lT, st4, on_act=True)
            # hT chunk = relu(W1^T vT + b1)
            for ch in range(NH):
                ps = psum_mm.tile([P, QB], F32, tag="mm")
                for ci in range(NC):
                    nc.tensor.matmul(
                        ps[:],
                        lhsT=w1_bf[:, ci, ch * P : (ch + 1) * P],
                        rhs=vT[:, ci, :],
                        start=(ci == 0),
                        stop=(ci == NC - 1),
                    )
                nc.scalar.activation(
                    out=hT[:, ch, :],
                    in_=ps[:],
                    func=AF.Relu,
                    bias=b1_sb[:, ch : ch + 1],
                    scale=1.0,
                )
            # K^T / Q^T chunk columns
            for dst, srcT, wgt, bias in ((kT, aT, wk_bf, bk_sb), (qT, lT, wq_bf, bq_sb)):
                for co in range(NC):
                    ps = psum_mm.tile([P, QB], F32, tag="mm")
                    for ci in range(NC):
                        nc.tensor.matmul(
                            ps[:],
                            lhsT=wgt[:, ci, co * P : (co + 1) * P],
                            rhs=srcT[:, ci, :],
                            start=(ci == 0),
                            stop=(ci == NC - 1),
                        )
                    nc.scalar.activation(
                        out=dst[:, co, sc * QB : (sc + 1) * QB],
                        in_=ps[:],
                        func=AF.Identity,
                        bias=bias[:, co : co + 1],
                        scale=1.0,
                    )
            # V chunk rows (natural [s, d])
            for st4 in range(TPC):
                st = sc * TPC + st4
                ps = psum_mm.tile([P, D], F32, tag="mm")
                for ci in range(NC):
                    nc.tensor.matmul(
                        ps[:],
                        lhsT=aT[:, ci, st4 * P : (st4 + 1) * P],
                        rhs=wv_bf[:, ci, :],
                        start=(ci == 0),
                        stop=(ci == NC - 1),
                    )
                nc.vector.tensor_add(v_sb[:, st, 0:D], ps[:], bv_bc[:])
            # w chunk = sigmoid(hT.T W2 + b2) = 0.5 + 0.5*tanh(0.5*(x+b2))
            for st4 in range(TPC):
                st = sc * TPC + st4
                psw = psum_w.tile([P, 1], F32, tag="small")
                for ch in range(NH):
                    nc.tensor.matmul(
                        psw[:],
                        lhsT=hT[:, ch, st4 * P : (st4 + 1) * P],
                        rhs=w2_bf[:, ch : ch + 1],
                        start=(ch == 0),
                        stop=(ch == NH - 1),
                    )
                wt = stage.tile([P, 1], F32, tag="wt", bufs=2)
                nc.scalar.activation(
                    out=wt[:], in_=psw[:], func=AF.Tanh, bias=0.5 * b2val, scale=0.5
                )
                nc.vector.tensor_scalar(
                    out=w_sb[:, st : st + 1],
                    in0=wt[:],
                    scalar1=0.5,
                    scalar2=0.5,
                    op0=OP.mult,
                    op1=OP.add,
                )

    # fused_v = w * v (re-load v; overlaps with attention)
    for st in range(NS):
        vf = stage.tile([P, D], F32, tag="v_re", bufs=3)
        nc.sync.dma_start(out=vf[:], in_=v_t[st])
        ov = stage.tile([P, D], F32, tag="out_v", bufs=3)
        nc.vector.tensor_scalar_mul(out=ov[:], in0=vf[:], scalar1=w_sb[:, st : st + 1])
        nc.scalar.dma_start(out=out[st, :, 2 * D : 3 * D], in_=ov[:])

    # ---- attention ----
    with (
        tc.tile_pool(name="ppool", bufs=3) as ppool,
        tc.tile_pool(name="psum_att", bufs=2, space="PSUM") as psum_att,
    ):
        for qb in range(NQB):
            pT = ppool.tile([P, NS, QB], BF16, tag="pT")
            # P^T[k, q] = exp(scale * (K^T.T Q^T))
            for kt in range(NS):
                ps = psum_mm.tile([P, QB], F32, tag="mm")
                for ci in range(NC):
                    nc.tensor.matmul(
                        ps[:],
                        lhsT=kT[:, ci, kt * P : (kt + 1) * P],
                        rhs=qT[:, ci, qb * QB : (qb + 1) * QB],
                        start=(ci == 0),
                        stop=(ci == NC - 1),
                    )
                nc.scalar.activation(
                    out=pT[:, kt, :], in_=ps[:], func=AF.Exp, scale=SCALE
                )
            # [O | r] = P [V | 1], accumulated over k tiles; epilogue per q-tile
            for qt in range(TPC):
                qi = qb * TPC + qt
                pso1 = psum_att.tile([P, N1], F32, tag="o1")
                pso2 = psum_att.tile([P, N2], F32, tag="o2")
                for kt in range(NS):
                    nc.tensor.matmul(
                        pso1[:],
                        lhsT=pT[:, kt, qt * P : (qt + 1) * P],
                        rhs=v_sb[:, kt, 0:N1],
                        start=(kt == 0),
                        stop=(kt == NS - 1),
                    )
                    nc.tensor.matmul(
                        pso2[:],
                        lhsT=pT[:, kt, qt * P : (qt + 1) * P],
                        rhs=v_sb[:, kt, N1:DV],
                        start=(kt == 0),
                        stop=(kt == NS - 1),
                    )
                rinv = stage.tile([P, 1], F32, tag="rinv", bufs=2)
                nc.vector.reciprocal(rinv[:], pso2[:, N2 - 1 : N2])
                t = stage.tile([P, D], F32, tag="t_l", bufs=2)
                nc.vector.tensor_scalar(
                    out=t[:, 0:N1],
                    in0=pso1[:],
                    scalar1=rinv[:],
                    scalar2=gl,
                    op0=OP.mult,
                    op1=OP.mult,
                )
                nc.vector.tensor_scalar(
                    out=t[:, N1:D],
                    in0=pso2[:, 0 : N2 - 1],
                    scalar1=rinv[:],
                    scalar2=gl,
                    op0=OP.mult,
                    op1=OP.mult,
                )
                lf = stage.tile([P, D], F32, tag="l_re", bufs=3)
                nc.sync.dma_start(out=lf[:], in_=l_t[qi])
                lsc = stage.tile([P, D], F32, tag="lsc", bufs=2)
                nc.scalar.mul(lsc[:], lf[:], 2.0 - gl)
                ol = stage.tile([P, D], F32, tag="out_l", bufs=3)
                nc.vector.tensor_add(ol[:], t[:], lsc[:])
                nc.gpsimd.dma_start(out=out[qi, :, 0:D], in_=ol[:])

    ctx.close()


def _execute(inputs, trace=False, **run_kwargs):
    a = np.ascontiguousarray(np.asarray(inputs["a"], dtype=np.float32))
    v = np.ascontiguousarray(np.asarray(inputs["v"], dtype=np.float32))
    l = np.ascontiguousarray(np.asarray(inputs["l"], dtype=np.float32))
    Wq = np.asarray(inputs["Wq"], dtype=np.float32)
    Wk = np.asarray(inputs["Wk"], dtype=np.float32)
    Wv = np.asarray(inputs["Wv"], dtype=np.float32)
    W1 = np.asarray(inputs["W1"], dtype=np.float32)
    W2 = np.asarray(inputs["W2"], dtype=np.float32)
    bq = np.asarray(inputs["bq"], dtype=np.float32)
    bk = np.asarray(inputs["bk"], dtype=np.float32)
    bv = np.asarray(inputs["bv"], dtype=np.float32)
    b1 = np.asarray(inputs["b1"], dtype=np.float32)
    b2 = np.asarray(inputs["b2"], dtype=np.float32)
    alpha_a = float(np.asarray(inputs["alpha_a"]))
    alpha_l = float(np.asarray(inputs["alpha_l"]))

    gl = float(1.0 / (1.0 + math.exp(-alpha_l)))
    ga = float(1.0 / (1.0 + math.exp(-alpha_a)))
    b2val = float(b2.reshape(-1)[0])

    nc = build_kernel(gl, ga, b2val)

    shared = {
        "wq": np.ascontiguousarray(Wq.reshape(NC, P, D)),
        "wk": np.ascontiguousarray(Wk.reshape(NC, P, D)),
        "wv": np.ascontiguousarray(Wv.reshape(NC, P, D)),
        "w1": np.ascontiguousarray(W1.reshape(NC, P, HID)),
        "w2": np.ascontiguousarray(W2.reshape(NH, P).T),
        "bq": np.ascontiguousarray(bq.reshape(NC, P).T),
        "bk": np.ascontiguousarray(bk.reshape(NC, P).T),
        "bv": np.ascontiguousarray(bv.reshape(1, D)),
        "b1": np.ascontiguousarray(b1.reshape(NH, P).T),
        "ident_in": np.eye(P, dtype=ml_dtypes.bfloat16),
    }
    in_maps = []
    for i in range(B):
        m = dict(shared)
        m["a_t"] = np.ascontiguousarray(a[i].reshape(NS, P, D))
        m["l_t"] = np.ascontiguousarray(l[i].reshape(NS, P, D))
        m["v_t"] = np.ascontiguousarray(v[i].reshape(NS, P, D))
        in_maps.append(m)

    res = run_bass_kernel_spmd(
        nc, in_maps, core_ids=list(range(B)), trace=trace, **run_kwargs
    )
    outs = [res.results[i]["out"].reshape(S, 3 * D) for i in range(B)]
    return np.stack(outs, axis=0).astype(np.float32), res


def kernel(**inputs) -> np.ndarray:
    out, _ = _execute(inputs, trace=False)
    return out


if __name__ == "__main__":
    print("kernel module OK")

